# revision 43
# baseline (speedup 1.0000x reference)
"""GCNNet (SimpleConv sum-aggr + global_mean_pool + 2-layer MLP) on 8 trn2 cores.

Math: out[g] = MLP(relu(sums[g] / max(counts[g],1)))
  sums[g,:]  = sum_e w_e * x[src_e,:] * [batch[dst_e]==g]
  counts[g]  = #{i : batch[i]==g}

Sharding (v3): by graph range (64 graphs per core) -> fully independent cores,
no collective.  Host canonicalizes each core's edge list (duplicate (src,
graph) cells coalesced, one row per distinct src) and quantizes the per-row
cell weights to uint8, folding each row's scale into that row's fp16 x copy.
On device the u8 coefficient windows are cast to fp16 during the SWDGE DMA
(halving their HBM traffic vs fp16) while the x windows stream on the HWDGE
queue.  Each window is one PE matmul with the coefficient block as the
STATIONARY operand: accT[64, 96] += C_w[128,64].T @ x_w[128,96] (f32 PSUM).
Node counts per graph come from 0/1 layer matrices contracted against ones.
A PE transpose flips accT||counts to [97, 64] for the tiny-MLP epilogue.
"""

import numpy as np

N_NODES = 50000
N_EDGES = 800000
D_FEAT = 96
D_HID = 10
N_GRAPHS = 512
CORES = 8
GPC = N_GRAPHS // CORES         # 64 graphs per core
P = 128

_nc_cache = {}


def _chunks(tot_w):
    """window chunks: big for DMA efficiency, small at the end for fast drain."""
    sizes = []
    rem = tot_w
    first = min(32, rem)
    sizes.append(first)
    rem -= first
    while rem > 88:
        sizes.append(64)
        rem -= 64
    if rem > 24:
        sizes.append(rem - 24)
        rem = 24
    for s in (12, 8, 4):
        if rem >= s:
            sizes.append(s)
            rem -= s
    if rem:
        sizes.append(rem)
    out = []
    w = 0
    for n in sizes:
        out.append((w, n))
        w += n
    return out


def _build_nc(tot_w, n_layers):
    import concourse.mybir as mybir
    import concourse.tile as tile
    from concourse import bacc

    f32 = mybir.dt.float32
    f16 = mybir.dt.float16
    u8 = mybir.dt.uint8
    G = GPC
    D = D_FEAT
    L = n_layers

    nc = bacc.Bacc(
        "TRN2",
        target_bir_lowering=False,
        debug=False,
        num_devices=CORES,
    )

    # fused per-window layout: [x row bytes (96 fp16 = 192B) | u8 cells (64B)]
    WB = 2 * D + G
    xc_d = nc.dram_tensor("xc", [P, tot_w * WB], u8, kind="ExternalInput")
    cm_d = nc.dram_tensor("cm", [P, L * G], f16, kind="ExternalInput")
    eye_d = nc.dram_tensor("eye", [G, G], f32, kind="ExternalInput")
    w1_d = nc.dram_tensor("w1", [D, D_HID], f32, kind="ExternalInput")
    b1_d = nc.dram_tensor("b1", [D_HID, 1], f32, kind="ExternalInput")
    w2_d = nc.dram_tensor("w2", [D_HID, 1], f32, kind="ExternalInput")
    b2_d = nc.dram_tensor("b2", [1, 1], f32, kind="ExternalInput")
    out_d = nc.dram_tensor("out", [1, G], f32, kind="ExternalOutput")

    chunks = _chunks(tot_w)

    with tile.TileContext(nc) as tc:
        with (
            tc.tile_pool(name="const", bufs=1) as cp,
            tc.tile_pool(name="cu8", bufs=6) as cu8p,
            tc.tile_pool(name="cw", bufs=6) as cwp,
            tc.tile_pool(name="psum", bufs=1, space="PSUM") as pp,
        ):
            accT_ps = pp.tile([G, D], f32, tag="accT")
            cntT_ps = pp.tile([G, 1], f32, tag="cntT")

            ones_t = cp.tile([P, 1], f16, tag="ones")
            nc.vector.memset(ones_t[:], 1.0)
            ones10_t = cp.tile([1, D_HID], f32, tag="ones10")
            nc.vector.memset(ones10_t[:], 1.0)

            qi = 0
            for c, (w0, nw) in enumerate(chunks):
                xc_t = cu8p.tile([P, 64 * WB], u8, tag="xc")
                ct = cwp.tile([P, 64 * G], f16, tag="cw")
                # split chunk into quarters: finer DMA->cast->matmul pipelining
                # shrinks how far the cast+matmul chain trails the DMA stream
                nparts = 4 if nw >= 16 else 2
                step = (nw + nparts - 1) // nparts
                bounds = [min(i * step, nw) for i in range(nparts + 1)]
                for h0, h1 in zip(bounds[:-1], bounds[1:]):
                    if h1 <= h0:
                        continue
                    dma_eng = (nc.sync, nc.scalar)[qi % 2]
                    qi += 1
                    dma_eng.dma_start(
                        out=xc_t[:, h0 * WB : h1 * WB],
                        in_=xc_d[:, (w0 + h0) * WB : (w0 + h1) * WB],
                    )
                    nc.vector.tensor_copy(
                        out=ct[:, h0 * G : h1 * G].rearrange(
                            "p (w b) -> p w b", b=G
                        ),
                        in_=xc_t[:, h0 * WB : h1 * WB].rearrange(
                            "p (w b) -> p w b", b=WB
                        )[:, :, 2 * D : WB],
                    )
                if c == 0:
                    # tiny consts ride behind chunk 0; count matmuls and the
                    # reciprocal chain run while the window stream flows
                    cm_t = cp.tile([P, L * G], f16, tag="cm")
                    nc.sync.dma_start(out=cm_t[:], in_=cm_d[:, :])
                    eye_t = cp.tile([G, G], f32, tag="eye")
                    nc.sync.dma_start(out=eye_t[:], in_=eye_d[:, :])
                    w1_t = cp.tile([D, D_HID], f32, tag="w1")
                    nc.sync.dma_start(out=w1_t[:], in_=w1_d[:, :])
                    b1_t = cp.tile([D_HID, 1], f32, tag="b1")
                    nc.sync.dma_start(out=b1_t[:], in_=b1_d[:, :])
                    w2_t = cp.tile([D_HID, 1], f32, tag="w2")
                    nc.sync.dma_start(out=w2_t[:], in_=w2_d[:, :])
                    b2_t = cp.tile([1, 1], f32, tag="b2")
                    nc.sync.dma_start(out=b2_t[:], in_=b2_d[:, :])
                if c == 1:
                    for l in range(L):
                        nc.tensor.matmul(
                            cntT_ps[:, :],
                            lhsT=cm_t[:, l * G : (l + 1) * G],
                            rhs=ones_t[:],
                            start=(l == 0),
                            stop=(l == L - 1),
                        )
                    # recip/rb only depend on counts: precompute off the tail
                    cntT_sb = cp.tile([G, 1], f32, tag="cnts")
                    nc.vector.tensor_copy(out=cntT_sb[:], in_=cntT_ps[:, :])
                    cntr_ps = pp.tile([1, G], f32, tag="cntr")
                    nc.tensor.transpose(cntr_ps[:, :], cntT_sb[:, :], eye_t[:])
                    cmax = cp.tile([1, G], f32, tag="cmax")
                    nc.vector.tensor_scalar_max(cmax[:], cntr_ps[:, :], 1.0)
                    recip = cp.tile([1, G], f32, tag="recip")
                    nc.vector.reciprocal(recip[:], cmax[:])
                    rb_ps = pp.tile([D_HID, G], f32, tag="rb")
                    nc.tensor.matmul(
                        rb_ps[:, :],
                        lhsT=ones10_t[:],
                        rhs=recip[:],
                        start=True,
                        stop=True,
                    )
                    rb_sb = cp.tile([D_HID, G], f32, tag="rbs")
                    nc.vector.tensor_copy(out=rb_sb[:, :], in_=rb_ps[:, :])
                for lw in range(nw):
                    w = w0 + lw
                    nc.tensor.matmul(
                        accT_ps[:, :],
                        lhsT=ct[:, lw * G : (lw + 1) * G],
                        rhs=xc_t[:, lw * WB : lw * WB + 2 * D].bitcast(f16),
                        start=(w == 0),
                        stop=(w == tot_w - 1),
                    )

            # relu(accT) then PE-transpose to [96, 64]
            a64_sb = cp.tile([G, D], f32, tag="a64")
            nc.vector.tensor_scalar_max(a64_sb[:, :], accT_ps[:, :], 0.0)
            fin_ps = pp.tile([D, G], f32, tag="fin")
            nc.tensor.transpose(fin_ps[:, :], a64_sb[:, :], eye_t[:])
            fin_sb = cp.tile([D, G], f32, tag="fins")
            nc.vector.tensor_copy(out=fin_sb[:, :], in_=fin_ps[:, :])

            # epilogue: relu already applied; positive 1/count scale commutes
            b_ps = pp.tile([D_HID, G], f32, tag="b")
            nc.tensor.matmul(
                b_ps[:, :], lhsT=w1_t[:], rhs=fin_sb[:D, :], start=True, stop=True
            )
            z_sb = cp.tile([D_HID, G], f32, tag="z")
            nc.vector.tensor_tensor(
                z_sb[:], b_ps[:, :], rb_sb[:], mybir.AluOpType.mult
            )
            nc.vector.tensor_scalar(
                out=z_sb[:],
                in0=z_sb[:],
                scalar1=b1_t[:],
                scalar2=0.0,
                op0=mybir.AluOpType.add,
                op1=mybir.AluOpType.max,
            )

            o_ps = pp.tile([1, G], f32, tag="o")
            nc.tensor.matmul(o_ps[:, :], lhsT=w2_t[:], rhs=z_sb[:], start=True, stop=True)
            o_sb = cp.tile([1, G], f32, tag="os")
            nc.vector.tensor_scalar(
                out=o_sb[:],
                in0=o_ps[:, :],
                scalar1=b2_t[:],
                scalar2=None,
                op0=mybir.AluOpType.add,
            )
            nc.sync.dma_start(out=out_d[:, :], in_=o_sb[:])

    nc.compile()
    return nc


def _occurrence_ranks(key):
    """rank of each element within its equal-key group (0-based), stable."""
    order = np.argsort(key, kind="stable")
    sk = key[order]
    n = len(sk)
    if n == 0:
        return np.zeros(0, np.int64)
    starts = np.r_[0, np.flatnonzero(np.diff(sk)) + 1]
    lens = np.diff(np.r_[starts, n])
    ranks_sorted = np.arange(n) - np.repeat(starts, lens)
    ranks = np.empty(n, np.int64)
    ranks[order] = ranks_sorted
    return ranks


def prepare_inputs(x, edge_index, edge_attr, batch, W1, b1, W2, b2):
    """Host-side reformatting (placement + sparse canonicalization only)."""
    G = GPC
    D = D_FEAT

    x = np.asarray(x, np.float32)
    src = np.asarray(edge_index[0], np.int64)
    dst = np.asarray(edge_index[1], np.int64)
    w = np.asarray(edge_attr, np.float32)
    batch = np.asarray(batch, np.int64)
    g = batch[dst]

    core = g // G
    per_core = []
    max_rows = 0
    # node range per core: batch is sorted
    node_bounds = np.searchsorted(batch, np.arange(CORES + 1) * G)
    n_layers = 1
    ranks_all = []
    for k in range(CORES):
        m = core == k
        sk_ = src[m]
        gk = (g[m] - k * G).astype(np.int64)
        wk = w[m].astype(np.float64)
        # coalesce duplicate (src, graph) cells; one cell per (src, g)
        cell_key = sk_ * G + gk
        uniq_cells, inv = np.unique(cell_key, return_inverse=True)
        w_cell = np.bincount(inv, weights=wk).astype(np.float32)
        src_c = uniq_cells // G
        g_c = uniq_cells % G
        # one row per distinct src
        uniq, row_of_cell = np.unique(src_c, return_inverse=True)
        max_rows = max(max_rows, len(uniq))
        per_core.append((uniq, row_of_cell, g_c, w_cell))

        n0, n1 = node_bounds[k], node_bounds[k + 1]
        bk = batch[n0:n1] - k * G
        pk = np.arange(n1 - n0) % P
        ranks = _occurrence_ranks(pk * G + bk)
        ranks_all.append((pk, ranks, bk))
        n_layers = max(n_layers, int(ranks.max(initial=-1)) + 1)

    tot_w = max(1, -(-max_rows // P))
    assert n_layers <= 8, n_layers

    in_maps = []
    for k in range(CORES):
        uniq, row_of_cell, g_c, w_cell = per_core[k]
        nrows = len(uniq)
        # per-row u8 quantization; the row scale is folded into the x row
        s_row = np.zeros(nrows, np.float32)
        np.maximum.at(s_row, row_of_cell, w_cell)
        s_row = np.where(s_row > 0, s_row, 1.0) / 255.0
        u = np.clip(np.rint(w_cell / s_row[row_of_cell]), 0, 255).astype(np.uint8)

        cu = np.zeros((P, tot_w, G), np.uint8)
        cu[row_of_cell % P, row_of_cell // P, g_c] = u

        xk = np.zeros((tot_w * P, D), np.float16)
        xk[:nrows] = (x[uniq] * s_row[:, None]).astype(np.float16)
        xw = np.ascontiguousarray(
            xk.reshape(tot_w, P, D).transpose(1, 0, 2)
        )  # [P, tot_w, D] fp16

        # fused per-window layout: [x bytes (192) | u8 cells (64)]
        WB = 2 * D + G
        xc = np.empty((P, tot_w, WB), np.uint8)
        xc[:, :, : 2 * D] = xw.view(np.uint8).reshape(P, tot_w, 2 * D)
        xc[:, :, 2 * D :] = cu
        xc = xc.reshape(P, tot_w * WB)

        pk, ranks, bk = ranks_all[k]
        cm = np.zeros((P, n_layers * G), np.float16)
        cm[pk, ranks * G + bk] = 1.0

        in_maps.append(
            {
                "xc": xc,
                "cm": cm,
                "eye": np.eye(G, dtype=np.float32),
                "w1": np.asarray(W1, np.float32).reshape(D, D_HID),
                "b1": np.asarray(b1, np.float32).reshape(D_HID, 1),
                "w2": np.asarray(W2, np.float32).reshape(D_HID, 1),
                "b2": np.asarray(b2, np.float32).reshape(1, 1),
            }
        )
    return in_maps, tot_w, n_layers


def get_nc(tot_w, n_layers):
    key = (tot_w, n_layers)
    if key not in _nc_cache:
        _nc_cache[key] = _build_nc(tot_w, n_layers)
    return _nc_cache[key]


def assemble(res):
    out = np.concatenate(
        [
            np.asarray(res.results[k]["out"], np.float32).reshape(GPC)
            for k in range(CORES)
        ]
    )
    return out.reshape(N_GRAPHS, 1)


def kernel(**inputs):
    from concourse import bass_utils

    in_maps, tot_w, n_layers = prepare_inputs(**inputs)
    nc = get_nc(tot_w, n_layers)
    res = bass_utils.run_bass_kernel_spmd(nc, in_maps, core_ids=list(range(CORES)))
    return assemble(res)


# revision 44
# speedup vs baseline: 1.0437x; 1.0437x over previous
"""GCNNet (SimpleConv sum-aggr + global_mean_pool + 2-layer MLP) on 8 trn2 cores.

Math: out[g] = MLP(relu(sums[g] / max(counts[g],1)))
  sums[g,:]  = sum_e w_e * x[src_e,:] * [batch[dst_e]==g]
  counts[g]  = #{i : batch[i]==g}

Sharding (v3): by graph range (64 graphs per core) -> fully independent cores,
no collective.  Host canonicalizes each core's edge list (duplicate (src,
graph) cells coalesced, one row per distinct src) and quantizes the per-row
cell weights to uint8, folding each row's scale into that row's fp16 x copy.
On device the u8 coefficient windows are cast to fp16 during the SWDGE DMA
(halving their HBM traffic vs fp16) while the x windows stream on the HWDGE
queue.  Each window is one PE matmul with the coefficient block as the
STATIONARY operand: accT[64, 96] += C_w[128,64].T @ x_w[128,96] (f32 PSUM).
Node counts per graph come from 0/1 layer matrices contracted against ones.
A PE transpose flips accT||counts to [97, 64] for the tiny-MLP epilogue.
"""

import numpy as np

N_NODES = 50000
N_EDGES = 800000
D_FEAT = 96
D_HID = 10
N_GRAPHS = 512
CORES = 8
GPC = N_GRAPHS // CORES         # 64 graphs per core
P = 128

_nc_cache = {}


def _chunks(tot_w):
    """window chunks: big for DMA efficiency, small at the end for fast drain."""
    sizes = []
    rem = tot_w
    first = min(32, rem)
    sizes.append(first)
    rem -= first
    while rem > 88:
        sizes.append(64)
        rem -= 64
    if rem > 24:
        sizes.append(rem - 24)
        rem = 24
    for s in (12, 8, 4):
        if rem >= s:
            sizes.append(s)
            rem -= s
    if rem:
        sizes.append(rem)
    out = []
    w = 0
    for n in sizes:
        out.append((w, n))
        w += n
    return out


def _build_nc(tot_w, n_layers):
    import concourse.mybir as mybir
    import concourse.tile as tile
    from concourse import bacc

    f32 = mybir.dt.float32
    f16 = mybir.dt.float16
    u8 = mybir.dt.uint8
    G = GPC
    D = D_FEAT
    L = n_layers

    nc = bacc.Bacc(
        "TRN2",
        target_bir_lowering=False,
        debug=False,
        num_devices=CORES,
    )

    # fused per-window layout: [x row bytes (96 fp16 = 192B) | u8 cells (64B)]
    WB = 2 * D + G
    xc_d = nc.dram_tensor("xc", [P, tot_w * WB], u8, kind="ExternalInput")
    cm_d = nc.dram_tensor("cm", [P, L * G], f16, kind="ExternalInput")
    eye_d = nc.dram_tensor("eye", [G, G], f32, kind="ExternalInput")
    w1_d = nc.dram_tensor("w1", [D, D_HID], f32, kind="ExternalInput")
    b1_d = nc.dram_tensor("b1", [D_HID, 1], f32, kind="ExternalInput")
    w2_d = nc.dram_tensor("w2", [D_HID, 1], f32, kind="ExternalInput")
    b2_d = nc.dram_tensor("b2", [1, 1], f32, kind="ExternalInput")
    out_d = nc.dram_tensor("out", [1, G], f32, kind="ExternalOutput")

    chunks = _chunks(tot_w)

    with tile.TileContext(nc) as tc:
        with (
            tc.tile_pool(name="const", bufs=1) as cp,
            tc.tile_pool(name="cu8", bufs=6) as cu8p,
            tc.tile_pool(name="cw", bufs=6) as cwp,
            tc.tile_pool(name="psum", bufs=1, space="PSUM") as pp,
        ):
            accT_ps = pp.tile([G, D], f32, tag="accT")
            cntT_ps = pp.tile([G, 1], f32, tag="cntT")

            ones_t = cp.tile([P, 1], f16, tag="ones")
            nc.vector.memset(ones_t[:], 1.0)
            ones10_t = cp.tile([1, D_HID], f32, tag="ones10")
            nc.vector.memset(ones10_t[:], 1.0)

            qi = 0
            for c, (w0, nw) in enumerate(chunks):
                xc_t = cu8p.tile([P, 64 * WB], u8, tag="xc")
                ct = cwp.tile([P, 64 * G], f16, tag="cw")
                # split chunk into halves: finer DMA->cast->matmul pipelining;
                # casts alternate between the vector and scalar engines
                nh = (nw + 1) // 2
                for h0, h1 in ((0, nh), (nh, nw)):
                    if h1 <= h0:
                        continue
                    dma_eng = (nc.sync, nc.scalar)[qi % 2]
                    qi += 1
                    dma_eng.dma_start(
                        out=xc_t[:, h0 * WB : h1 * WB],
                        in_=xc_d[:, (w0 + h0) * WB : (w0 + h1) * WB],
                    )
                    nc.vector.tensor_copy(
                        out=ct[:, h0 * G : h1 * G].rearrange(
                            "p (w b) -> p w b", b=G
                        ),
                        in_=xc_t[:, h0 * WB : h1 * WB].rearrange(
                            "p (w b) -> p w b", b=WB
                        )[:, :, 2 * D : WB],
                    )
                if c == 0:
                    # tiny consts ride behind chunk 0; count matmuls and the
                    # reciprocal chain run while the window stream flows
                    cm_t = cp.tile([P, L * G], f16, tag="cm")
                    nc.sync.dma_start(out=cm_t[:], in_=cm_d[:, :])
                    eye_t = cp.tile([G, G], f32, tag="eye")
                    nc.sync.dma_start(out=eye_t[:], in_=eye_d[:, :])
                    w1_t = cp.tile([D, D_HID], f32, tag="w1")
                    nc.sync.dma_start(out=w1_t[:], in_=w1_d[:, :])
                    b1_t = cp.tile([D_HID, 1], f32, tag="b1")
                    nc.sync.dma_start(out=b1_t[:], in_=b1_d[:, :])
                    w2_t = cp.tile([D_HID, 1], f32, tag="w2")
                    nc.sync.dma_start(out=w2_t[:], in_=w2_d[:, :])
                    b2_t = cp.tile([1, 1], f32, tag="b2")
                    nc.sync.dma_start(out=b2_t[:], in_=b2_d[:, :])
                if c == 1:
                    for l in range(L):
                        nc.tensor.matmul(
                            cntT_ps[:, :],
                            lhsT=cm_t[:, l * G : (l + 1) * G],
                            rhs=ones_t[:],
                            start=(l == 0),
                            stop=(l == L - 1),
                        )
                    # recip/rb only depend on counts: precompute off the tail
                    cntT_sb = cp.tile([G, 1], f32, tag="cnts")
                    nc.vector.tensor_copy(out=cntT_sb[:], in_=cntT_ps[:, :])
                    cntr_ps = pp.tile([1, G], f32, tag="cntr")
                    nc.tensor.transpose(cntr_ps[:, :], cntT_sb[:, :], eye_t[:])
                    cmax = cp.tile([1, G], f32, tag="cmax")
                    nc.vector.tensor_scalar_max(cmax[:], cntr_ps[:, :], 1.0)
                    recip = cp.tile([1, G], f32, tag="recip")
                    nc.vector.reciprocal(recip[:], cmax[:])
                    rb_ps = pp.tile([D_HID, G], f32, tag="rb")
                    nc.tensor.matmul(
                        rb_ps[:, :],
                        lhsT=ones10_t[:],
                        rhs=recip[:],
                        start=True,
                        stop=True,
                    )
                    rb_sb = cp.tile([D_HID, G], f32, tag="rbs")
                    nc.vector.tensor_copy(out=rb_sb[:, :], in_=rb_ps[:, :])
                for lw in range(nw):
                    w = w0 + lw
                    nc.tensor.matmul(
                        accT_ps[:, :],
                        lhsT=ct[:, lw * G : (lw + 1) * G],
                        rhs=xc_t[:, lw * WB : lw * WB + 2 * D].bitcast(f16),
                        start=(w == 0),
                        stop=(w == tot_w - 1),
                    )

            # relu(accT) then PE-transpose to [96, 64]
            a64_sb = cp.tile([G, D], f32, tag="a64")
            nc.vector.tensor_scalar_max(a64_sb[:, :], accT_ps[:, :], 0.0)
            fin_ps = pp.tile([D, G], f32, tag="fin")
            nc.tensor.transpose(fin_ps[:, :], a64_sb[:, :], eye_t[:])
            fin_sb = cp.tile([D, G], f32, tag="fins")
            nc.vector.tensor_copy(out=fin_sb[:, :], in_=fin_ps[:, :])

            # epilogue: relu already applied; positive 1/count scale commutes
            b_ps = pp.tile([D_HID, G], f32, tag="b")
            nc.tensor.matmul(
                b_ps[:, :], lhsT=w1_t[:], rhs=fin_sb[:D, :], start=True, stop=True
            )
            z_sb = cp.tile([D_HID, G], f32, tag="z")
            nc.vector.tensor_tensor(
                z_sb[:], b_ps[:, :], rb_sb[:], mybir.AluOpType.mult
            )
            nc.vector.tensor_scalar(
                out=z_sb[:],
                in0=z_sb[:],
                scalar1=b1_t[:],
                scalar2=0.0,
                op0=mybir.AluOpType.add,
                op1=mybir.AluOpType.max,
            )

            o_ps = pp.tile([1, G], f32, tag="o")
            nc.tensor.matmul(o_ps[:, :], lhsT=w2_t[:], rhs=z_sb[:], start=True, stop=True)
            o_sb = cp.tile([1, G], f32, tag="os")
            nc.vector.tensor_scalar(
                out=o_sb[:],
                in0=o_ps[:, :],
                scalar1=b2_t[:],
                scalar2=None,
                op0=mybir.AluOpType.add,
            )
            nc.sync.dma_start(out=out_d[:, :], in_=o_sb[:])

    nc.compile()
    return nc


def _occurrence_ranks(key):
    """rank of each element within its equal-key group (0-based), stable."""
    order = np.argsort(key, kind="stable")
    sk = key[order]
    n = len(sk)
    if n == 0:
        return np.zeros(0, np.int64)
    starts = np.r_[0, np.flatnonzero(np.diff(sk)) + 1]
    lens = np.diff(np.r_[starts, n])
    ranks_sorted = np.arange(n) - np.repeat(starts, lens)
    ranks = np.empty(n, np.int64)
    ranks[order] = ranks_sorted
    return ranks


def prepare_inputs(x, edge_index, edge_attr, batch, W1, b1, W2, b2):
    """Host-side reformatting (placement + sparse canonicalization only)."""
    G = GPC
    D = D_FEAT

    x = np.asarray(x, np.float32)
    src = np.asarray(edge_index[0], np.int64)
    dst = np.asarray(edge_index[1], np.int64)
    w = np.asarray(edge_attr, np.float32)
    batch = np.asarray(batch, np.int64)
    g = batch[dst]

    core = g // G
    per_core = []
    max_rows = 0
    # node range per core: batch is sorted
    node_bounds = np.searchsorted(batch, np.arange(CORES + 1) * G)
    n_layers = 1
    ranks_all = []
    for k in range(CORES):
        m = core == k
        sk_ = src[m]
        gk = (g[m] - k * G).astype(np.int64)
        wk = w[m].astype(np.float64)
        # coalesce duplicate (src, graph) cells; one cell per (src, g)
        cell_key = sk_ * G + gk
        uniq_cells, inv = np.unique(cell_key, return_inverse=True)
        w_cell = np.bincount(inv, weights=wk).astype(np.float32)
        src_c = uniq_cells // G
        g_c = uniq_cells % G
        # one row per distinct src
        uniq, row_of_cell = np.unique(src_c, return_inverse=True)
        max_rows = max(max_rows, len(uniq))
        per_core.append((uniq, row_of_cell, g_c, w_cell))

        n0, n1 = node_bounds[k], node_bounds[k + 1]
        bk = batch[n0:n1] - k * G
        pk = np.arange(n1 - n0) % P
        ranks = _occurrence_ranks(pk * G + bk)
        ranks_all.append((pk, ranks, bk))
        n_layers = max(n_layers, int(ranks.max(initial=-1)) + 1)

    tot_w = max(1, -(-max_rows // P))
    assert n_layers <= 8, n_layers

    in_maps = []
    for k in range(CORES):
        uniq, row_of_cell, g_c, w_cell = per_core[k]
        nrows = len(uniq)
        # per-row u8 quantization; the row scale is folded into the x row
        s_row = np.zeros(nrows, np.float32)
        np.maximum.at(s_row, row_of_cell, w_cell)
        s_row = np.where(s_row > 0, s_row, 1.0) / 255.0
        u = np.clip(np.rint(w_cell / s_row[row_of_cell]), 0, 255).astype(np.uint8)

        cu = np.zeros((P, tot_w, G), np.uint8)
        cu[row_of_cell % P, row_of_cell // P, g_c] = u

        xk = np.zeros((tot_w * P, D), np.float16)
        xk[:nrows] = (x[uniq] * s_row[:, None]).astype(np.float16)
        xw = np.ascontiguousarray(
            xk.reshape(tot_w, P, D).transpose(1, 0, 2)
        )  # [P, tot_w, D] fp16

        # fused per-window layout: [x bytes (192) | u8 cells (64)]
        WB = 2 * D + G
        xc = np.empty((P, tot_w, WB), np.uint8)
        xc[:, :, : 2 * D] = xw.view(np.uint8).reshape(P, tot_w, 2 * D)
        xc[:, :, 2 * D :] = cu
        xc = xc.reshape(P, tot_w * WB)

        pk, ranks, bk = ranks_all[k]
        cm = np.zeros((P, n_layers * G), np.float16)
        cm[pk, ranks * G + bk] = 1.0

        in_maps.append(
            {
                "xc": xc,
                "cm": cm,
                "eye": np.eye(G, dtype=np.float32),
                "w1": np.asarray(W1, np.float32).reshape(D, D_HID),
                "b1": np.asarray(b1, np.float32).reshape(D_HID, 1),
                "w2": np.asarray(W2, np.float32).reshape(D_HID, 1),
                "b2": np.asarray(b2, np.float32).reshape(1, 1),
            }
        )
    return in_maps, tot_w, n_layers


def get_nc(tot_w, n_layers):
    key = (tot_w, n_layers)
    if key not in _nc_cache:
        _nc_cache[key] = _build_nc(tot_w, n_layers)
    return _nc_cache[key]


def assemble(res):
    out = np.concatenate(
        [
            np.asarray(res.results[k]["out"], np.float32).reshape(GPC)
            for k in range(CORES)
        ]
    )
    return out.reshape(N_GRAPHS, 1)


def kernel(**inputs):
    from concourse import bass_utils

    in_maps, tot_w, n_layers = prepare_inputs(**inputs)
    nc = get_nc(tot_w, n_layers)
    res = bass_utils.run_bass_kernel_spmd(nc, in_maps, core_ids=list(range(CORES)))
    return assemble(res)
